# revision 42
# baseline (speedup 1.0000x reference)
"""DifferentialGPT forward on 8 TRN2 NeuronCores (Bass/Tile).

DP2 over batch x TP4 (4 heads, 512 MLP pairs, 12800 vocab cols per core).
bf16 AllReduce (groups of 4) after out_proj and c_proj, split into T-halves
and software-pipelined against compute, including across the layer boundary
(layer l+1 half-0 stats/QKV/attention overlap layer l's last AllReduce).

Residual xT stays f32; a bf16 shadow xB feeds all projection matmuls so
every matmul is bf16 (FWL weight loads, halved weight DMA). All 512-wide
accumulations (QKV/out_proj/MLP) rotate through four 1-bank PSUM tiles;
attention score matmuls are emitted 4-stacked across 32-row PE bands, V is
stored 128-padded per head so AV matmuls get fast weight loads, and the
rmsnorm per-token-rstd transposes use 4 contract-1 matmuls off the std row.
QKV/W1 projection fills are emitted before the rmsnorm-stats matmul so the
PE does not wait on the Square->sum->sqrt->recip chain; Squares and PSUM
drains are split scalar/vector. ln_f emits both halves' stats matmuls
before the scalar/vector chains; lm_head weight groups are prefetched
before ln_f. Weight DMAs ride the Activation queue, stores the SP queue,
AllReduces and resid loads the Pool queue.
"""
import math
import numpy as np

V, BLK, C, H, L = 50257, 1024, 1024, 16, 4
B, T = 2, 1024
HS = C // H          # 64
D = HS // 2          # 32
EPS = 1e-5
N_CORES = 8
TP = 4
HPC = H // TP        # 4
VSH = 12800
HIDS = 4 * C // TP // 2   # 512
NCB = C // 128       # 8
NT = T // 512        # 2
NSB = T // 128       # 8
NGB = HIDS // 128    # 4

_BUILT = {}
_MARKS = []


def _build(rep_count=1, no_collective=False, ar_mode="half"):
    # ar_mode: "half"  - one AllReduce per T-half (default)
    #          "full"  - one AllReduce per sublayer over full T
    #          "nowait"- half ARs issued, but residual reads the un-reduced
    #                    input (WRONG results; timing diagnostic only)
    from concourse import bass, mybir, bacc
    import concourse.tile as tile

    F32 = mybir.dt.float32
    F32R = mybir.dt.float32r
    BF16 = mybir.dt.bfloat16
    AF = mybir.ActivationFunctionType
    ALU = mybir.AluOpType

    nc = bacc.Bacc("TRN2", target_bir_lowering=False, debug=False,
                   num_devices=N_CORES)
    _MARKS.clear()

    def mark(label):
        _MARKS.append((label, int(nc.get_next_instruction_name()[2:])))
    for _cv in (EPS,):
        _ct = nc.alloc_sbuf_tensor(f"const-float32-{_cv}", [128, 1], F32)
        nc.gpsimd.memset(_ct.ap(), _cv)
        nc.const_aps.aps[(F32, _cv)] = _ct.ap()
    nc.all_engine_barrier()

    def EIN(name, shape, dt):
        return nc.dram_tensor(name, shape, dt, kind="ExternalInput")

    x0_e = EIN("x0", [C, T], F32R)
    x0b_e = EIN("x0b", [C, T], BF16)
    wq_e = EIN("wq", [L, C, HPC * HS], BF16)
    wk_e = EIN("wk", [L, C, HPC * HS], BF16)
    wv_e = EIN("wv", [L, C, HPC * HS], BF16)
    ow_e = EIN("ow", [L, HPC * HS, C], BF16)
    w1_e = EIN("w1", [L, C, 2 * HIDS], BF16)
    w2_e = EIN("w2", [L, HIDS, C], BF16)
    neglam_e = EIN("neglam", [L, HPC], F32)
    masks_e = EIN("masks", [128, 4, 2048], BF16)
    wteT_e = EIN("wteT", [C, VSH], BF16)
    logits_e = nc.dram_tensor("logits", [T, VSH], BF16,
                              kind="ExternalOutput")

    GROUPS = [[0, 1, 2, 3], [4, 5, 6, 7]]

    with tile.TileContext(nc) as tc:
      with (
        tc.tile_pool(name="persist", bufs=1) as pp,
        tc.tile_pool(name="dram", bufs=1, space="DRAM") as dram,
      ):
        masks = pp.tile([128, 4, 1024], BF16, name="masks")
        nc.gpsimd.dma_start(out=masks[:], in_=masks_e[:, :, 0:1024])
        neglam = pp.tile([128, L, HPC], F32, name="neglam")
        nc.gpsimd.dma_start(
            out=neglam[:],
            in_=bass.AP(tensor=neglam_e, offset=0,
                        ap=[[0, 128], [HPC, L], [1, HPC]]))
        ones_f = pp.tile([128, 128], F32, name="ones_f")
        nc.vector.memset(ones_f[:], 1.0)
        ones128 = pp.tile([128, 128], F32R, name="ones128")
        nc.vector.tensor_copy(ones128[:], ones_f[:])
        ones128b = pp.tile([128, 128], BF16, name="ones128b")
        nc.vector.tensor_copy(ones128b[:], ones_f[:])
        ones_cb = ones128b[:, 0:1]

        for _rep in range(rep_count):
          xpool = tc.alloc_tile_pool(name=f"xpool{_rep}", bufs=1)
          xT = xpool.tile([128, NCB, T], F32R, name="xT")
          xB = xpool.tile([128, NCB, T], BF16, name="xB")
          for tt in range(NT):
              tsl = slice(tt * 512, (tt + 1) * 512)
              eng = nc.gpsimd if tt == 0 else nc.sync
              eng.dma_start(
                  out=xB[:, :, tsl],
                  in_=x0b_e.ap()[:, tsl].rearrange("(a p) t -> p a t",
                                                   p=128))
              eng.dma_start(
                  out=xT[:, :, tsl],
                  in_=x0_e.ap()[:, tsl].rearrange("(a p) t -> p a t",
                                                  p=128))

          with tc.tile_pool(name=f"ps{_rep}", bufs=1, space="PSUM") as sp:

            def psum_big(name):
                return sp.tile([128, 1024], F32, tag="sc", bufs=2,
                               name=name)

            _rot = [0]

            def psum_rot(name):
                i = _rot[0] % 4
                _rot[0] += 1
                return sp.tile([128, 512], F32, tag=f"av{i}", name=name)


            # x^2 (split scalar/vector), for rmsnorm stats of half tt
            def stats_sq(tt, tag):
                tsl = slice(tt * 512, (tt + 1) * 512)
                sq = xpool.tile([128, NCB, 512], BF16, tag="sq", bufs=2,
                                name=f"sq{tag}{tt}")
                for cb in range(NCB):
                    if cb % 2 == 0:
                        nc.scalar.activation(sq[:, cb, :], xT[:, cb, tsl],
                                             AF.Square)
                    else:
                        nc.vector.tensor_mul(sq[:, cb, :], xT[:, cb, tsl],
                                             xT[:, cb, tsl])
                return sq

            # sum(x^2) matmul -> sqrt -> (emitted separately from recip)
            def stats_ssq(sq, tag, tt):
                big = psum_big(f"ssq{tag}{tt}")
                ssq_b = big[:, 0:512]
                for cb in range(NCB):
                    nc.tensor.matmul(ssq_b, ones128b[:], sq[:, cb, :],
                                     start=(cb == 0),
                                     stop=(cb == NCB - 1))
                return big

            def stats_std(pool, big, tag, tt):
                std_b = pool.tile([128, 512], F32, tag=f"sdb{tag}",
                                  name=f"sdb{tag}{tt}")
                nc.scalar.activation(std_b[:], big[:, 0:512], AF.Sqrt,
                                     scale=1.0 / C, bias=EPS)
                return std_b

            def stats_rstd_b(pool, std_b, tag, tt):
                rstd_b = pool.tile([128, 512], F32, tag=f"rsb{tag}",
                                   name=f"rsb{tag}{tt}")
                nc.vector.reciprocal_approx_fast(out=rstd_b[:],
                                                 in_=std_b[:])
                return rstd_b

            pend_h1_final = [None]

            for l in range(L):
                at = tc.alloc_tile_pool(name=f"at{_rep}_{l}", bufs=1)
                wq = at.tile([128, NCB, HPC * HS], BF16, name=f"wq{l}")
                wk = at.tile([128, NCB, HPC * HS], BF16, name=f"wk{l}")
                wv = at.tile([128, NCB, HPC * HS], BF16, name=f"wv{l}")
                ow = at.tile([128, (HPC * HS) // 128, C], BF16,
                             name=f"ow{l}")
                for t_sb, t_e in ((wq, wq_e), (wk, wk_e), (wv, wv_e),
                                  (ow, ow_e)):
                    nc.scalar.dma_start(
                        out=t_sb[:],
                        in_=t_e.ap()[l].rearrange("(a p) m -> p a m",
                                                  p=128))

                qT = at.tile([128, 2, T], BF16, name=f"qT{l}")
                kT = at.tile([128, 2, T], BF16, name=f"kT{l}")
                vN = at.tile([128, NSB, HPC * 128], BF16,
                             name=f"vN{l}")
                vN4 = vN[:].rearrange("p s (h e) -> p s h e", e=128)
                oT = at.tile([128, 2, T], BF16, name=f"oT{l}")

                def qkv_half(tt):
                    tsl = slice(tt * 512, (tt + 1) * 512)
                    # x^2 first (scalar/vector), then q/k fills (tensor,
                    # independent of stats), then the stats matmul, then
                    # v fills, then the std transpose, then all drains.
                    sq = stats_sq(tt, "a")
                    pqs = []
                    for dst, w_sb in ((qT, wq), (kT, wk)):
                        for mb in range(2):
                            pq = psum_rot("pq")[:, 0:512]
                            for cb in range(NCB):
                                nc.tensor.matmul(
                                    pq,
                                    w_sb[:, cb, mb * 128:(mb + 1) * 128],
                                    xB[:, cb, tsl],
                                    start=(cb == 0),
                                    stop=(cb == NCB - 1))
                            pqs.append((dst, mb, pq))
                    big = stats_ssq(sq, "a", tt)
                    std_b = stats_std(at, big, "a", tt)
                    pvs = []
                    for sb4 in range(4):
                        sb = tt * 4 + sb4
                        nc.vector.tensor_copy(vN4[:, sb, :, HS:HS + 1],
                                              ones_f[:, 0:HPC])
                        pv = psum_rot("pv")
                        for cb in range(NCB):
                            nc.tensor.matmul(
                                pv[:, 0:HPC * HS],
                                xB[:, cb, sb * 128:(sb + 1) * 128],
                                wv[:, cb, :],
                                start=(cb == 0), stop=(cb == NCB - 1))
                        pvs.append((sb, sb4, pv))
                    # transpose std row -> per-token std cols (psum bank B
                    # of the stats tile), f32 contract-1 matmuls
                    for sb4 in range(4):
                        nc.tensor.matmul(
                            big[:, 512 + sb4:513 + sb4],
                            std_b[0:1, sb4 * 128:(sb4 + 1) * 128],
                            ones_f[0:1, 0:1],
                            start=True, stop=True)
                    # drains (vector queue order: rstd_b, q/k, rstd_c, v)
                    rstd_b = stats_rstd_b(at, std_b, "a", tt)
                    for dst, mb, pq in pqs:
                        nc.vector.tensor_mul(dst[:, mb, tsl], pq,
                                             rstd_b[:])
                    rstd_c = at.tile([128, 8], F32, tag="rsca",
                                     name=f"rsca{tt}")
                    nc.vector.reciprocal_approx_fast(
                        out=rstd_c[:, 0:4], in_=big[:, 512:516])
                    for sb, sb4, pv in pvs:
                        nc.vector.tensor_scalar_mul(
                            vN4[:, sb, :, 0:HS],
                            pv[:, 0:HPC * HS].rearrange(
                                "p (h e) -> p h e", e=HS),
                            rstd_c[:, sb4:sb4 + 1])

                def attn_w(tt, w):
                    n_sb = (tt + 1) * 4
                    tsl = slice(tt * 512, (tt + 1) * 512)
                    avp = [sp.tile([128, 512], F32, tag=f"av{i}",
                                   name=f"av{i}") for i in range(4)]
                    pend_av = []

                    def do_av(sb, Ps):
                        first, last = (sb == 0), (sb == n_sb - 1)
                        for hp in range(2):
                            hh = 2 * w + hp
                            for qq in range(2):
                                q = 2 * hp + qq
                                nc.tensor.matmul(
                                    avp[q][:, :],
                                    vN[:, sb,
                                       hh * 128:(hh + 1) * 128],
                                    Ps[hp][:,
                                           qq * 512:(qq + 1) * 512],
                                    start=first, stop=last)

                    for sb in range(n_sb):
                        scps = [psum_big(f"sc{hp}") for hp in range(2)]
                        for hp in range(2):
                            for qq in range(2):
                                q = 2 * hp + qq
                                nc.tensor.matmul(
                                    scps[hp][:,
                                             qq * 512:(qq + 1) * 512],
                                    kT[32 * q:32 * q + 32, w,
                                       sb * 128:(sb + 1) * 128],
                                    qT[32 * q:32 * q + 32, w, tsl],
                                    start=True, stop=True,
                                    tile_position=(32 * q, 0))
                        Ps = []
                        for hp in range(2):
                            P = at.tile([128, 1024], BF16,
                                        tag=f"P{hp}", bufs=3,
                                        name=f"P{hp}")
                            nc.scalar.activation(P[:], scps[hp][:],
                                                 AF.Exp)
                            m = sb - tt * 4
                            if m >= 0:
                                nc.vector.tensor_mul(
                                    P[:], P[:], masks[:, m, :])
                            Ps.append(P)
                        pend_av.append((sb, Ps))
                        if len(pend_av) > 2:
                            do_av(*pend_av.pop(0))
                    while pend_av:
                        do_av(*pend_av.pop(0))

                    # combine dual softmax -> oT
                    zr = at.tile([128, 512], F32R, tag="zr",
                                 name="zr")
                    bcs = []
                    for q in range(4):
                        nc.vector.tensor_copy(zr[64:65, :],
                                              avp[q][64:65, :])
                        bcp = psum_big(f"bc{q}")[:, 0:512]
                        nc.tensor.matmul(
                            bcp, ones128[64:65, :], zr[64:65, :],
                            start=True, stop=True,
                            tile_position=(64, 0))
                        bc = at.tile([128, 512], F32,
                                     tag=f"bcs{q % 2}", bufs=1,
                                     name=f"bcsb{q}")
                        nc.vector.reciprocal_approx_fast(
                            out=bc[:], in_=bcp)
                        bcs.append(bc)
                    scr = at.tile([128, 512], F32, tag="scr",
                                  name="scr")
                    oshift = at.tile([128, 512], BF16, tag="osh",
                                     bufs=2, name="oshift")
                    for hl in range(2):
                        h = 2 * w + hl
                        dst = (oT[0:64, w, tsl] if hl == 0
                               else oshift[0:64, :])
                        nc.vector.tensor_mul(
                            scr[0:64, :], avp[2 * hl + 1][0:64, :],
                            bcs[2 * hl + 1][0:64, :])
                        nc.vector.tensor_mul(
                            dst, avp[2 * hl][0:64, :],
                            bcs[2 * hl][0:64, :])
                        nc.vector.scalar_tensor_tensor(
                            out=dst, in0=scr[0:64, :],
                            scalar=neglam[0:64, l, h:h + 1],
                            in1=dst,
                            op0=ALU.mult, op1=ALU.add)
                        if hl == 1:
                            nc.sync.dma_start(
                                out=oT[64:128, w, tsl],
                                in_=oshift[0:64, :])

                if ar_mode == "full":
                    _ai = dram.tile([C, T], BF16, tag="ariAF",
                                    name=f"ariAF_{l}")
                    _ao = dram.tile([C, T], BF16, tag="aroAF",
                                    name=f"aroAF_{l}")
                    arA_in = [_ai[:, 0:512], _ai[:, 512:1024]]
                    arA_out = [_ao[:, 0:512], _ao[:, 512:1024]]
                    arA_full = (_ai, _ao)
                else:
                    arA_in = [dram.tile([C, 512], BF16, tag=f"ariA{tt}",
                                        name=f"ariA{tt}_{l}")[:]
                              for tt in range(2)]
                    arA_out = [dram.tile([C, 512], BF16,
                                         tag=f"aroA{tt}",
                                         name=f"aroA{tt}_{l}")[:]
                               for tt in range(2)]
                    arA_full = None

                def op_half(tt):
                    # tt=1 feeds the exposed AR(A1): keep its drains off the
                    # Exp-congested scalar queue and split stores across two
                    # DMA queues so the collective issues sooner.
                    tsl = slice(tt * 512, (tt + 1) * 512)
                    for mb in range(NCB):
                        po = psum_rot("po")[:, 0:512]
                        for kb in range(2):
                            nc.tensor.matmul(
                                po,
                                ow[:, kb, mb * 128:(mb + 1) * 128],
                                oT[:, kb, tsl],
                                start=(kb == 0), stop=(kb == 1))
                        yst = at.tile([128, 512], BF16, tag="yst",
                                      bufs=4, name="yst")
                        if tt == 0 and mb % 2 == 0:
                            nc.scalar.copy(yst[:], po)
                        else:
                            nc.vector.tensor_copy(yst[:], po)
                        eng = (nc.sync if (tt == 0 or mb % 2 == 0)
                               else nc.gpsimd)
                        eng.dma_start(
                            out=arA_in[tt][mb * 128:(mb + 1) * 128, :],
                            in_=yst[:])
                    if no_collective:
                        nc.sync.dma_start(out=arA_out[tt],
                                          in_=arA_in[tt])
                    elif ar_mode == "full":
                        if tt == 1:
                            nc.gpsimd.collective_compute(
                                "AllReduce", ALU.add,
                                replica_groups=GROUPS,
                                ins=[arA_full[0].opt()],
                                outs=[arA_full[1].opt()])
                    else:
                        nc.gpsimd.collective_compute(
                            "AllReduce", ALU.add, replica_groups=GROUPS,
                            ins=[arA_in[tt].opt()],
                            outs=[arA_out[tt].opt()])

                # ---- emission schedule ----
                mark(f"qkv0_{l}")
                qkv_half(0)
                mark(f"attn0_{l}")
                attn_w(0, 0)
                attn_w(0, 1)
                mark(f"op0_{l}")
                op_half(0)                      # -> AR A(0)
                mark(f"fin_{l}")
                if pend_h1_final[0] is not None:
                    pend_h1_final[0]()          # resid M(l-1, h1)
                    pend_h1_final[0] = None

                # ======== MLP pool + AR buffers ========
                ml = tc.alloc_tile_pool(name=f"ml{_rep}_{l}", bufs=1,
                                        side="right")
                w1 = ml.tile([128, NCB, 2 * HIDS], BF16, name=f"w1{l}")
                w2 = ml.tile([128, NGB, C], BF16, name=f"w2{l}")
                nc.scalar.dma_start(
                    out=w1[:],
                    in_=w1_e.ap()[l].rearrange("(a p) m -> p a m",
                                               p=128))
                nc.scalar.dma_start(
                    out=w2[:],
                    in_=w2_e.ap()[l].rearrange("(a p) m -> p a m",
                                               p=128))
                if ar_mode == "full":
                    _mi = dram.tile([C, T], BF16, tag="ariMF",
                                    name=f"ariMF_{l}")
                    _mo = dram.tile([C, T], BF16, tag="aroMF",
                                    name=f"aroMF_{l}")
                    arM_in = [_mi[:, 0:512], _mi[:, 512:1024]]
                    arM_out = [_mo[:, 0:512], _mo[:, 512:1024]]
                    arM_full = (_mi, _mo)
                else:
                    arM_in = [dram.tile([C, 512], BF16, tag=f"ariM{tt}",
                                        name=f"ariM{tt}_{l}")[:]
                              for tt in range(2)]
                    arM_out = [dram.tile([C, 512], BF16,
                                         tag=f"aroM{tt}",
                                         name=f"aroM{tt}_{l}")[:]
                               for tt in range(2)]
                    arM_full = None
                gsb = [ml.tile([128, NGB, 512], BF16, name=f"gsb{i}")
                       for i in range(2)]
                asb = [ml.tile([128, NGB, 512], BF16, name=f"asb{i}")
                       for i in range(2)]

                def resid_half(tt, ar_out, tag, ar_in_=None):
                    if ar_mode == "nowait" and ar_in_ is not None:
                        ar_out = ar_in_
                    tsl = slice(tt * 512, (tt + 1) * 512)
                    for cb in range(NCB):
                        yf = ml.tile([128, 512], BF16, tag=f"yf{tag}",
                                     bufs=4, name=f"yf{tag}")
                        (nc.gpsimd if cb % 2 == 0
                         else nc.sync).dma_start(
                            out=yf[:],
                            in_=ar_out[cb * 128:(cb + 1) * 128, :])
                        nc.vector.tensor_add(xT[:, cb, tsl],
                                             xT[:, cb, tsl], yf[:])
                        if cb % 2 == 0:
                            nc.scalar.copy(xB[:, cb, tsl],
                                           xT[:, cb, tsl])
                        else:
                            nc.vector.tensor_copy(xB[:, cb, tsl],
                                                  xT[:, cb, tsl])

                def mlp_pu(tt, gsb_, asb_):
                    tsl = slice(tt * 512, (tt + 1) * 512)
                    sq = stats_sq(tt, "m")
                    pus = []

                    def pu_fill(gb):
                        pu = psum_rot(f"pu{gb}")[:, 0:512]
                        for cb in range(NCB):
                            nc.tensor.matmul(
                                pu,
                                w1[:, cb, gb * 128:(gb + 1) * 128],
                                xB[:, cb, tsl],
                                start=(cb == 0), stop=(cb == NCB - 1))
                        pus.append((gb, pu))

                    for gb in range(NGB):
                        pu_fill(gb)
                    big = stats_ssq(sq, "m", tt)
                    std_b = stats_std(ml, big, "m", tt)
                    for gb in range(NGB, 2 * NGB):
                        pu_fill(gb)
                    rstd2b = stats_rstd_b(ml, std_b, "m", tt)
                    for gb, pu in pus:
                        if gb < NGB:
                            nc.vector.tensor_mul(gsb_[:, gb, :], pu,
                                                 rstd2b[:])
                        else:
                            nc.vector.tensor_mul(asb_[:, gb - NGB, :],
                                                 pu, rstd2b[:])

                def mlp_act(tt, gsb_, asb_):
                    for gb in range(NGB):
                        nc.scalar.activation(asb_[:, gb, :],
                                             asb_[:, gb, :], AF.Silu)
                        nc.vector.tensor_mul(gsb_[:, gb, :],
                                             asb_[:, gb, :],
                                             gsb_[:, gb, :])

                def mlp_pz(tt, gsb_):
                    for mb in range(NCB):
                        pz = psum_rot(f"pz{mb}")[:, 0:512]
                        for kb in range(NGB):
                            nc.tensor.matmul(
                                pz,
                                w2[:, kb, mb * 128:(mb + 1) * 128],
                                gsb_[:, kb, :],
                                start=(kb == 0), stop=(kb == NGB - 1))
                        zst = ml.tile([128, 512], BF16, tag="zst",
                                      bufs=4, name="zst")
                        if tt == 0 or mb % 2 == 0:
                            nc.vector.tensor_copy(zst[:], pz)
                        else:
                            nc.scalar.copy(zst[:], pz)
                        eng = (nc.sync if (tt == 1 or mb % 2 == 0)
                               else nc.gpsimd)
                        eng.dma_start(
                            out=arM_in[tt][mb * 128:(mb + 1) * 128, :],
                            in_=zst[:])
                    if no_collective:
                        nc.sync.dma_start(out=arM_out[tt],
                                          in_=arM_in[tt])
                    elif ar_mode == "full":
                        if tt == 1:
                            nc.gpsimd.collective_compute(
                                "AllReduce", ALU.add,
                                replica_groups=GROUPS,
                                ins=[arM_full[0].opt()],
                                outs=[arM_full[1].opt()])
                    else:
                        nc.gpsimd.collective_compute(
                            "AllReduce", ALU.add, replica_groups=GROUPS,
                            ins=[arM_in[tt].opt()],
                            outs=[arM_out[tt].opt()])

                def make_final(ml_pool_, arM_out1_, arM_in1_):
                    def fin():
                        tsl = slice(512, 1024)
                        for cb in range(NCB):
                            zf = ml_pool_.tile([128, 512], BF16,
                                               tag="yfm", bufs=4,
                                               name="zf")
                            src_ = (arM_in1_ if ar_mode == "nowait"
                                    else arM_out1_)
                            (nc.gpsimd if cb % 2 == 0
                             else nc.sync).dma_start(
                                out=zf[:],
                                in_=src_[cb * 128:
                                         (cb + 1) * 128, :])
                            nc.vector.tensor_add(xT[:, cb, tsl],
                                                 xT[:, cb, tsl],
                                                 zf[:])
                            if cb % 2 == 0:
                                nc.scalar.copy(xB[:, cb, tsl],
                                               xT[:, cb, tsl])
                            else:
                                nc.vector.tensor_copy(xB[:, cb, tsl],
                                                      xT[:, cb, tsl])
                        ml_pool_.release()
                    return fin

                mark(f"qkv1_{l}")
                qkv_half(1)
                mark(f"attn1a_{l}")
                attn_w(1, 0)
                mark(f"attn1b_{l}")
                attn_w(1, 1)
                mark(f"op1_{l}")
                op_half(1)                      # -> AR A(1)
                at.release()
                mark(f"ra0_{l}")
                resid_half(0, arA_out[0], "a", arA_in[0])
                mark(f"mlp0_{l}")
                mlp_pu(0, gsb[0], asb[0])
                mlp_act(0, gsb[0], asb[0])
                mlp_pz(0, gsb[0])               # -> AR M(0)
                mark(f"ra1_{l}")
                resid_half(1, arA_out[1], "a", arA_in[1])
                mark(f"mlp1_{l}")
                mlp_pu(1, gsb[1], asb[1])
                mlp_act(1, gsb[1], asb[1])
                mlp_pz(1, gsb[1])               # -> AR M(1)
                mark(f"rm0_{l}")
                resid_half(0, arM_out[0], "m", arM_in[0])

                pend_h1_final[0] = make_final(ml, arM_out[1],
                                              arM_in[1])

            if pend_h1_final[0] is not None:
                pend_h1_final[0]()
                pend_h1_final[0] = None

            mark("lnf")
            # ---------------- ln_f -> xc (bf16) ----------------
            hd = tc.alloc_tile_pool(name=f"hd{_rep}", bufs=1)
            xc = hd.tile([128, NCB, T], BF16, name="xc")
            rstd_tok = hd.tile([128, 8], F32, name="rstdtok")
            hw = tc.alloc_tile_pool(name=f"hw{_rep}", bufs=1)
            NVT = VSH // 512            # 25
            VG = 2
            n_groups = (NVT + VG - 1) // VG
            wts = {}

            def load_wt(g):
                vts = list(range(g * VG, min((g + 1) * VG, NVT)))
                nv = len(vts)
                wt = hw.tile([128, NCB, VG * 512], BF16, tag="wt",
                             bufs=3, name=f"wt{g}")
                nc.scalar.dma_start(
                    out=wt[:, :, 0:nv * 512],
                    in_=wteT_e.ap()[:, vts[0] * 512:
                                    (vts[-1] + 1) * 512]
                        .rearrange("(a p) v -> p a v", p=128))
                wts[g] = (wt, vts, nv)

            load_wt(0)
            load_wt(1)

            hd0 = tc.alloc_tile_pool(name=f"hd0{_rep}", bufs=1)
            bigs = []
            for tt in range(NT):
                tsl = slice(tt * 512, (tt + 1) * 512)
                big = psum_big("mu_ms")
                mu_b = big[:, 0:512]
                for cb in range(NCB):
                    nc.tensor.matmul(mu_b, ones128[:], xT[:, cb, tsl],
                                     start=(cb == 0),
                                     stop=(cb == NCB - 1))
                sqf = xpool.tile([128, NCB, 512], BF16, tag="sq",
                                 bufs=2, name="sqf")
                for cb in range(NCB):
                    if cb % 2 == 0:
                        nc.scalar.activation(sqf[:, cb, :],
                                             xT[:, cb, tsl], AF.Square)
                    else:
                        nc.vector.tensor_mul(sqf[:, cb, :],
                                             xT[:, cb, tsl],
                                             xT[:, cb, tsl])
                ms_b = big[:, 512:1024]
                for cb in range(NCB):
                    nc.tensor.matmul(ms_b, ones128b[:], sqf[:, cb, :],
                                     start=(cb == 0),
                                     stop=(cb == NCB - 1))
                bigs.append(big)
            for tt in range(NT):
                tsl = slice(tt * 512, (tt + 1) * 512)
                big = bigs[tt]
                mu_b, ms_b = big[:, 0:512], big[:, 512:1024]
                negmu = hd0.tile([128, 512], F32, tag=f"negmu{tt}",
                                 name="negmu")
                nc.scalar.activation(negmu[:], mu_b, AF.Copy,
                                     scale=-1.0 / C)
                mom2 = hd0.tile([128, 512], F32, tag=f"mom2{tt}",
                                name="mom2")
                nc.scalar.activation(mom2[:], ms_b, AF.Copy,
                                     scale=1.0 / C)
                mu2 = hd0.tile([128, 512], F32, tag=f"mu2{tt}",
                               name="mu2")
                nc.vector.tensor_mul(mu2[:], negmu[:], negmu[:])
                var = hd0.tile([128, 512], F32, tag=f"var{tt}",
                               name="var")
                nc.vector.tensor_sub(var[:], mom2[:], mu2[:])
                stdf = hd0.tile([128, 512], F32, tag=f"stdf{tt}",
                                name="stdf")
                nc.scalar.activation(stdf[:], var[:], AF.Sqrt, bias=EPS)
                # transpose std row -> per-token cols (bank B is free
                # once mom2 has read it); rstd applied at lm_head drain
                for tb4 in range(4):
                    nc.tensor.matmul(
                        big[:, 512 + tb4:513 + tb4],
                        stdf[0:1, tb4 * 128:(tb4 + 1) * 128],
                        ones_f[0:1, 0:1],
                        start=True, stop=True)
                nc.vector.reciprocal_approx_fast(
                    out=rstd_tok[:, tt * 4:tt * 4 + 4],
                    in_=big[:, 512:516])
                for cb in range(NCB):
                    nc.vector.tensor_add(xc[:, cb, tsl],
                                         xT[:, cb, tsl], negmu[:])
            hd0.release()

            mark("lmhead")
            # ---------------- lm_head (bf16) ----------------
            for g in range(n_groups):
                if g + 2 < n_groups:
                    load_wt(g + 2)
                wt, vts, nv = wts.pop(g)
                for tb in range(NSB):
                    po = 2 * (tb % 2)
                    lps = [sp.tile([128, 512], F32,
                                   tag=f"av{po + i}",
                                   name=f"lg{g}_{tb}_{i}")
                           for i in range(nv)]
                    for cb in range(NCB):
                        for i in range(nv):
                            nc.tensor.matmul(
                                lps[i][:],
                                xc[:, cb,
                                   tb * 128:(tb + 1) * 128],
                                wt[:, cb, i * 512:(i + 1) * 512],
                                start=(cb == 0),
                                stop=(cb == NCB - 1))
                    lo = hd.tile([128, VG * 512], BF16, tag="lo",
                                 bufs=3, name=f"lo{g}_{tb}")
                    for i in range(nv):
                        if i % 2 == 0:
                            nc.scalar.activation(
                                lo[:, i * 512:(i + 1) * 512],
                                lps[i][:], AF.Copy,
                                scale=rstd_tok[:, tb:tb + 1])
                        else:
                            nc.vector.tensor_scalar_mul(
                                lo[:, i * 512:(i + 1) * 512],
                                lps[i][:],
                                rstd_tok[:, tb:tb + 1])
                    nc.sync.dma_start(
                        out=logits_e[tb * 128:(tb + 1) * 128,
                                     vts[0] * 512:
                                     (vts[-1] + 1) * 512],
                        in_=lo[:, 0:nv * 512])
            hw.release()
            hd.release()

          xpool.release()

    nc.compile()
    return nc


def _prep_inputs(inputs):
    import ml_dtypes
    idx = np.asarray(inputs["idx"]).astype(np.int64)
    wte = np.asarray(inputs["wte"], np.float32)
    wpe = np.asarray(inputs["wpe"], np.float32)
    rms1 = np.asarray(inputs["rms1_w"], np.float32)
    rms2 = np.asarray(inputs["rms2_w"], np.float32)
    wq = np.asarray(inputs["wq"], np.float32)
    wk = np.asarray(inputs["wk"], np.float32)
    wv = np.asarray(inputs["wv"], np.float32)
    lq1 = np.asarray(inputs["lq1"], np.float32)
    lq2 = np.asarray(inputs["lq2"], np.float32)
    lk1 = np.asarray(inputs["lk1"], np.float32)
    lk2 = np.asarray(inputs["lk2"], np.float32)
    out_w = np.asarray(inputs["out_w"], np.float32)
    out_b = np.asarray(inputs["out_b"], np.float32)
    mlp_w1 = np.asarray(inputs["mlp_w1"], np.float32)
    mlp_b1 = np.asarray(inputs["mlp_b1"], np.float32)
    cproj_w = np.asarray(inputs["cproj_w"], np.float32)
    cproj_b = np.asarray(inputs["cproj_b"], np.float32)
    lnf_w = np.asarray(inputs["lnf_w"], np.float32)

    assert not (np.any(out_b) or np.any(mlp_b1) or np.any(cproj_b)), \
        "nonzero biases not supported by this kernel build"

    depth = np.arange(L, dtype=np.float32)
    lam_init = 0.8 - 0.6 * np.exp(-0.3 * (depth - 1.0))
    lam = (np.exp((lq1 * lk1).sum(-1)) - np.exp((lq2 * lk2).sum(-1))
           + lam_init[:, None])

    wteE = wte[:BLK] + wpe
    scale = 1.0 / math.sqrt(D)
    wq_f = wq * rms1[:, :, None, None] * scale
    wk_f = wk * rms1[:, :, None, None]
    wv_f = wv * rms1[:, :, None, None]
    w1_f = mlp_w1 * rms2[:, :, None]
    wteT_full = np.ascontiguousarray((wte * lnf_w[None, :]).T)

    jj = np.arange(512)[None, :]
    ppp = np.arange(128)[:, None]
    masks = np.zeros((128, 4, 2048), np.float32)
    for m in range(4):
        one = (jj >= (ppp + 128 * m)).astype(np.float32)
        masks[:, m, :] = np.tile(one, (1, 4))
    masks = masks.astype(ml_dtypes.bfloat16)

    in_maps = []
    for c in range(N_CORES):
        b, r = c // TP, c % TP
        hsl = slice(r * HPC, (r + 1) * HPC)
        x0 = np.ascontiguousarray(wteE[idx[b]].T)
        g0 = r * HIDS
        a0 = 2 * C + r * HIDS
        w1_s = np.concatenate(
            [w1_f[:, :, g0:g0 + HIDS], w1_f[:, :, a0:a0 + HIDS]],
            axis=2)
        w2_s = cproj_w[:, g0:g0 + HIDS, :]
        wteT_s = np.zeros((C, VSH), np.float32)
        lo, hi = r * VSH, min((r + 1) * VSH, V)
        if hi > lo:
            wteT_s[:, 0:hi - lo] = wteT_full[:, lo:hi]
        in_maps.append({
            "x0": x0,
            "x0b": x0.astype(ml_dtypes.bfloat16),
            "wq": np.ascontiguousarray(
                wq_f[:, :, hsl].reshape(L, C, HPC * HS)).astype(
                    ml_dtypes.bfloat16),
            "wk": np.ascontiguousarray(
                wk_f[:, :, hsl].reshape(L, C, HPC * HS)).astype(
                    ml_dtypes.bfloat16),
            "wv": np.ascontiguousarray(
                wv_f[:, :, hsl].reshape(L, C, HPC * HS)).astype(
                    ml_dtypes.bfloat16),
            "ow": np.ascontiguousarray(
                out_w.reshape(L, H, HS, C)[:, hsl].reshape(
                    L, HPC * HS, C)).astype(ml_dtypes.bfloat16),
            "w1": np.ascontiguousarray(w1_s).astype(ml_dtypes.bfloat16),
            "w2": np.ascontiguousarray(w2_s).astype(ml_dtypes.bfloat16),
            "neglam": np.ascontiguousarray(-lam[:, hsl]),
            "masks": masks,
            "wteT": wteT_s.astype(ml_dtypes.bfloat16),
        })
    return in_maps


def kernel(**inputs):
    from concourse.bass_utils import run_bass_kernel_spmd
    if "nc" not in _BUILT:
        _BUILT["nc"] = _build()
    nc = _BUILT["nc"]
    in_maps = _prep_inputs(inputs)
    res = run_bass_kernel_spmd(nc, in_maps, core_ids=list(range(N_CORES)))
    outs = []
    for b in range(B):
        parts = [res.results[b * TP + r]["logits"]
                 for r in range(TP)]
        outs.append(np.concatenate(parts, axis=1)[:, :V])
    return np.stack(outs, axis=0).astype(np.float32)


# revision 43
# speedup vs baseline: 1.0223x; 1.0223x over previous
"""DifferentialGPT forward on 8 TRN2 NeuronCores (Bass/Tile).

DP2 over batch x TP4 (4 heads, 512 MLP pairs, 12800 vocab cols per core).
bf16 AllReduce (groups of 4) after out_proj and c_proj, split into T-halves
and software-pipelined against compute, including across the layer boundary
(layer l+1 half-0 stats/QKV/attention overlap layer l's last AllReduce).

Residual xT stays f32; a bf16 shadow xB feeds all projection matmuls so
every matmul is bf16 (FWL weight loads, halved weight DMA). All 512-wide
accumulations (QKV/out_proj/MLP) rotate through four 1-bank PSUM tiles;
attention score matmuls are emitted 4-stacked across 32-row PE bands, V is
stored 128-padded per head so AV matmuls get fast weight loads, and the
rmsnorm per-token-rstd transposes use 4 contract-1 matmuls off the std row.
QKV/W1 projection fills are emitted before the rmsnorm-stats matmul so the
PE does not wait on the Square->sum->sqrt->recip chain; Squares and PSUM
drains are split scalar/vector. ln_f emits both halves' stats matmuls
before the scalar/vector chains; lm_head weight groups are prefetched
before ln_f. Weight DMAs ride the Activation queue, stores the SP queue,
AllReduces and resid loads the Pool queue.
"""
import math
import numpy as np

V, BLK, C, H, L = 50257, 1024, 1024, 16, 4
B, T = 2, 1024
HS = C // H          # 64
D = HS // 2          # 32
EPS = 1e-5
N_CORES = 8
TP = 4
HPC = H // TP        # 4
VSH = 12800
HIDS = 4 * C // TP // 2   # 512
NCB = C // 128       # 8
NT = T // 512        # 2
NSB = T // 128       # 8
NGB = HIDS // 128    # 4

_BUILT = {}
_MARKS = []


def _build(rep_count=1, no_collective=False, ar_mode="half"):
    # ar_mode: "half"  - one AllReduce per T-half (default)
    #          "full"  - one AllReduce per sublayer over full T
    #          "nowait"- half ARs issued, but residual reads the un-reduced
    #                    input (WRONG results; timing diagnostic only)
    from concourse import bass, mybir, bacc
    import concourse.tile as tile

    F32 = mybir.dt.float32
    F32R = mybir.dt.float32r
    BF16 = mybir.dt.bfloat16
    AF = mybir.ActivationFunctionType
    ALU = mybir.AluOpType

    nc = bacc.Bacc("TRN2", target_bir_lowering=False, debug=False,
                   num_devices=N_CORES)
    _MARKS.clear()

    def mark(label):
        _MARKS.append((label, int(nc.get_next_instruction_name()[2:])))
    for _cv in (EPS,):
        _ct = nc.alloc_sbuf_tensor(f"const-float32-{_cv}", [128, 1], F32)
        nc.gpsimd.memset(_ct.ap(), _cv)
        nc.const_aps.aps[(F32, _cv)] = _ct.ap()
    nc.all_engine_barrier()

    def EIN(name, shape, dt):
        return nc.dram_tensor(name, shape, dt, kind="ExternalInput")

    x0_e = EIN("x0", [C, T], F32R)
    x0b_e = EIN("x0b", [C, T], BF16)
    wq_e = EIN("wq", [L, C, HPC * HS], BF16)
    wk_e = EIN("wk", [L, C, HPC * HS], BF16)
    wv_e = EIN("wv", [L, C, HPC * HS], BF16)
    ow_e = EIN("ow", [L, HPC * HS, C], BF16)
    w1_e = EIN("w1", [L, C, 2 * HIDS], BF16)
    w2_e = EIN("w2", [L, HIDS, C], BF16)
    neglam_e = EIN("neglam", [L, HPC], F32)
    masks_e = EIN("masks", [128, 4, 2048], BF16)
    wteT_e = EIN("wteT", [C, VSH], BF16)
    logits_e = nc.dram_tensor("logits", [T, VSH], BF16,
                              kind="ExternalOutput")

    GROUPS = [[0, 1, 2, 3], [4, 5, 6, 7]]

    with tile.TileContext(nc) as tc:
      with (
        tc.tile_pool(name="persist", bufs=1) as pp,
        tc.tile_pool(name="dram", bufs=1, space="DRAM") as dram,
      ):
        masks = pp.tile([128, 4, 1024], BF16, name="masks")
        nc.gpsimd.dma_start(out=masks[:], in_=masks_e[:, :, 0:1024])
        neglam = pp.tile([128, L, HPC], F32, name="neglam")
        nc.gpsimd.dma_start(
            out=neglam[:],
            in_=bass.AP(tensor=neglam_e, offset=0,
                        ap=[[0, 128], [HPC, L], [1, HPC]]))
        ones_f = pp.tile([128, 128], F32, name="ones_f")
        nc.vector.memset(ones_f[:], 1.0)
        ones128 = pp.tile([128, 128], F32R, name="ones128")
        nc.vector.tensor_copy(ones128[:], ones_f[:])
        ones128b = pp.tile([128, 128], BF16, name="ones128b")
        nc.vector.tensor_copy(ones128b[:], ones_f[:])
        ones_cb = ones128b[:, 0:1]

        for _rep in range(rep_count):
          xpool = tc.alloc_tile_pool(name=f"xpool{_rep}", bufs=1)
          xT = xpool.tile([128, NCB, T], F32R, name="xT")
          xB = xpool.tile([128, NCB, T], BF16, name="xB")
          for tt in range(NT):
              tsl = slice(tt * 512, (tt + 1) * 512)
              eng = nc.gpsimd if tt == 0 else nc.sync
              eng.dma_start(
                  out=xB[:, :, tsl],
                  in_=x0b_e.ap()[:, tsl].rearrange("(a p) t -> p a t",
                                                   p=128))
              eng.dma_start(
                  out=xT[:, :, tsl],
                  in_=x0_e.ap()[:, tsl].rearrange("(a p) t -> p a t",
                                                  p=128))

          with tc.tile_pool(name=f"ps{_rep}", bufs=1, space="PSUM") as sp:

            def psum_big(name):
                return sp.tile([128, 1024], F32, tag="sc", bufs=2,
                               name=name)

            _rot = [0]

            def psum_rot(name):
                i = _rot[0] % 4
                _rot[0] += 1
                return sp.tile([128, 512], F32, tag=f"av{i}", name=name)


            # x^2 (split scalar/vector), for rmsnorm stats of half tt
            def stats_sq(tt, tag):
                tsl = slice(tt * 512, (tt + 1) * 512)
                sq = xpool.tile([128, NCB, 512], BF16, tag="sq", bufs=2,
                                name=f"sq{tag}{tt}")
                for cb in range(NCB):
                    if cb % 2 == 0:
                        nc.scalar.activation(sq[:, cb, :], xT[:, cb, tsl],
                                             AF.Square)
                    else:
                        nc.vector.tensor_mul(sq[:, cb, :], xT[:, cb, tsl],
                                             xT[:, cb, tsl])
                return sq

            # sum(x^2) matmul -> sqrt -> (emitted separately from recip)
            def stats_ssq(sq, tag, tt):
                big = psum_big(f"ssq{tag}{tt}")
                ssq_b = big[:, 0:512]
                for cb in range(NCB):
                    nc.tensor.matmul(ssq_b, ones128b[:], sq[:, cb, :],
                                     start=(cb == 0),
                                     stop=(cb == NCB - 1))
                return big

            def stats_std(pool, big, tag, tt):
                std_b = pool.tile([128, 512], F32, tag=f"sdb{tag}",
                                  name=f"sdb{tag}{tt}")
                nc.scalar.activation(std_b[:], big[:, 0:512], AF.Sqrt,
                                     scale=1.0 / C, bias=EPS)
                return std_b

            def stats_rstd_b(pool, std_b, tag, tt):
                rstd_b = pool.tile([128, 512], F32, tag=f"rsb{tag}",
                                   name=f"rsb{tag}{tt}")
                nc.vector.reciprocal_approx_fast(out=rstd_b[:],
                                                 in_=std_b[:])
                return rstd_b

            pend_h1_final = [None]

            for l in range(L):
                at = tc.alloc_tile_pool(name=f"at{_rep}_{l}", bufs=1)
                wq = at.tile([128, NCB, HPC * HS], BF16, name=f"wq{l}")
                wk = at.tile([128, NCB, HPC * HS], BF16, name=f"wk{l}")
                wv = at.tile([128, NCB, HPC * HS], BF16, name=f"wv{l}")
                ow = at.tile([128, (HPC * HS) // 128, C], BF16,
                             name=f"ow{l}")
                for t_sb, t_e in ((wq, wq_e), (wk, wk_e), (wv, wv_e),
                                  (ow, ow_e)):
                    nc.scalar.dma_start(
                        out=t_sb[:],
                        in_=t_e.ap()[l].rearrange("(a p) m -> p a m",
                                                  p=128))

                qT = at.tile([128, 2, T], BF16, name=f"qT{l}")
                kT = at.tile([128, 2, T], BF16, name=f"kT{l}")
                vN = at.tile([128, NSB, HPC * 128], BF16,
                             name=f"vN{l}")
                vN4 = vN[:].rearrange("p s (h e) -> p s h e", e=128)
                oT = at.tile([128, 2, T], BF16, name=f"oT{l}")

                def qkv_half(tt):
                    tsl = slice(tt * 512, (tt + 1) * 512)
                    # x^2 first (scalar/vector), then q/k fills (tensor,
                    # independent of stats), then the stats matmul, then
                    # v fills, then the std transpose, then all drains.
                    sq = stats_sq(tt, "a")
                    pqs = []
                    for mb in range(2):
                        for dst, w_sb in ((qT, wq), (kT, wk)):
                            pq = psum_rot("pq")[:, 0:512]
                            for cb in range(NCB):
                                nc.tensor.matmul(
                                    pq,
                                    w_sb[:, cb, mb * 128:(mb + 1) * 128],
                                    xB[:, cb, tsl],
                                    start=(cb == 0),
                                    stop=(cb == NCB - 1))
                            pqs.append((dst, mb, pq))
                    big = stats_ssq(sq, "a", tt)
                    std_b = stats_std(at, big, "a", tt)
                    pvs = []
                    for sb4 in range(4):
                        sb = tt * 4 + sb4
                        nc.vector.tensor_copy(vN4[:, sb, :, HS:HS + 1],
                                              ones_f[:, 0:HPC])
                        pv = psum_rot("pv")
                        for cb in range(NCB):
                            nc.tensor.matmul(
                                pv[:, 0:HPC * HS],
                                xB[:, cb, sb * 128:(sb + 1) * 128],
                                wv[:, cb, :],
                                start=(cb == 0), stop=(cb == NCB - 1))
                        pvs.append((sb, sb4, pv))
                    # transpose std row -> per-token std cols (psum bank B
                    # of the stats tile), f32 contract-1 matmuls
                    for sb4 in range(4):
                        nc.tensor.matmul(
                            big[:, 512 + sb4:513 + sb4],
                            std_b[0:1, sb4 * 128:(sb4 + 1) * 128],
                            ones_f[0:1, 0:1],
                            start=True, stop=True)
                    # drains (vector queue order: rstd_b, q/k, rstd_c, v)
                    rstd_b = stats_rstd_b(at, std_b, "a", tt)
                    for dst, mb, pq in pqs:
                        nc.vector.tensor_mul(dst[:, mb, tsl], pq,
                                             rstd_b[:])
                    rstd_c = at.tile([128, 8], F32, tag="rsca",
                                     name=f"rsca{tt}")
                    nc.vector.reciprocal_approx_fast(
                        out=rstd_c[:, 0:4], in_=big[:, 512:516])
                    for sb, sb4, pv in pvs:
                        nc.vector.tensor_scalar_mul(
                            vN4[:, sb, :, 0:HS],
                            pv[:, 0:HPC * HS].rearrange(
                                "p (h e) -> p h e", e=HS),
                            rstd_c[:, sb4:sb4 + 1])

                def attn_w(tt, w):
                    n_sb = (tt + 1) * 4
                    tsl = slice(tt * 512, (tt + 1) * 512)
                    avp = [sp.tile([128, 512], F32, tag=f"av{i}",
                                   name=f"av{i}") for i in range(4)]
                    pend_av = []

                    def do_av(sb, Ps):
                        first, last = (sb == 0), (sb == n_sb - 1)
                        for hp in range(2):
                            hh = 2 * w + hp
                            for qq in range(2):
                                q = 2 * hp + qq
                                nc.tensor.matmul(
                                    avp[q][:, :],
                                    vN[:, sb,
                                       hh * 128:(hh + 1) * 128],
                                    Ps[hp][:,
                                           qq * 512:(qq + 1) * 512],
                                    start=first, stop=last)

                    for sb in range(n_sb):
                        scps = [psum_big(f"sc{hp}") for hp in range(2)]
                        for hp in range(2):
                            for qq in range(2):
                                q = 2 * hp + qq
                                nc.tensor.matmul(
                                    scps[hp][:,
                                             qq * 512:(qq + 1) * 512],
                                    kT[32 * q:32 * q + 32, w,
                                       sb * 128:(sb + 1) * 128],
                                    qT[32 * q:32 * q + 32, w, tsl],
                                    start=True, stop=True,
                                    tile_position=(32 * q, 0))
                        Ps = []
                        for hp in range(2):
                            P = at.tile([128, 1024], BF16,
                                        tag=f"P{hp}", bufs=3,
                                        name=f"P{hp}")
                            nc.scalar.activation(P[:], scps[hp][:],
                                                 AF.Exp)
                            m = sb - tt * 4
                            if m >= 0:
                                nc.vector.tensor_mul(
                                    P[:], P[:], masks[:, m, :])
                            Ps.append(P)
                        pend_av.append((sb, Ps))
                        if len(pend_av) > 2:
                            do_av(*pend_av.pop(0))
                    while pend_av:
                        do_av(*pend_av.pop(0))

                    # combine dual softmax -> oT
                    zr = at.tile([128, 512], F32R, tag="zr",
                                 name="zr")
                    bcs = []
                    for q in range(4):
                        nc.vector.tensor_copy(zr[64:65, :],
                                              avp[q][64:65, :])
                        bcp = psum_big(f"bc{q}")[:, 0:512]
                        nc.tensor.matmul(
                            bcp, ones128[64:65, :], zr[64:65, :],
                            start=True, stop=True,
                            tile_position=(64, 0))
                        bc = at.tile([128, 512], F32,
                                     tag=f"bcs{q % 2}", bufs=1,
                                     name=f"bcsb{q}")
                        nc.vector.reciprocal_approx_fast(
                            out=bc[:], in_=bcp)
                        bcs.append(bc)
                    scr = at.tile([128, 512], F32, tag="scr",
                                  name="scr")
                    oshift = at.tile([128, 512], BF16, tag="osh",
                                     bufs=2, name="oshift")
                    for hl in range(2):
                        h = 2 * w + hl
                        dst = (oT[0:64, w, tsl] if hl == 0
                               else oshift[0:64, :])
                        nc.vector.tensor_mul(
                            scr[0:64, :], avp[2 * hl + 1][0:64, :],
                            bcs[2 * hl + 1][0:64, :])
                        nc.vector.tensor_mul(
                            dst, avp[2 * hl][0:64, :],
                            bcs[2 * hl][0:64, :])
                        nc.vector.scalar_tensor_tensor(
                            out=dst, in0=scr[0:64, :],
                            scalar=neglam[0:64, l, h:h + 1],
                            in1=dst,
                            op0=ALU.mult, op1=ALU.add)
                        if hl == 1:
                            nc.sync.dma_start(
                                out=oT[64:128, w, tsl],
                                in_=oshift[0:64, :])

                if ar_mode == "full":
                    _ai = dram.tile([C, T], BF16, tag="ariAF",
                                    name=f"ariAF_{l}")
                    _ao = dram.tile([C, T], BF16, tag="aroAF",
                                    name=f"aroAF_{l}")
                    arA_in = [_ai[:, 0:512], _ai[:, 512:1024]]
                    arA_out = [_ao[:, 0:512], _ao[:, 512:1024]]
                    arA_full = (_ai, _ao)
                else:
                    arA_in = [dram.tile([C, 512], BF16, tag=f"ariA{tt}",
                                        name=f"ariA{tt}_{l}")[:]
                              for tt in range(2)]
                    arA_out = [dram.tile([C, 512], BF16,
                                         tag=f"aroA{tt}",
                                         name=f"aroA{tt}_{l}")[:]
                               for tt in range(2)]
                    arA_full = None

                def op_half(tt):
                    # tt=1 feeds the exposed AR(A1): keep its drains off the
                    # Exp-congested scalar queue and split stores across two
                    # DMA queues so the collective issues sooner.
                    tsl = slice(tt * 512, (tt + 1) * 512)
                    for mb in range(NCB):
                        po = psum_rot("po")[:, 0:512]
                        for kb in range(2):
                            nc.tensor.matmul(
                                po,
                                ow[:, kb, mb * 128:(mb + 1) * 128],
                                oT[:, kb, tsl],
                                start=(kb == 0), stop=(kb == 1))
                        yst = at.tile([128, 512], BF16, tag="yst",
                                      bufs=4, name="yst")
                        if tt == 0 and mb % 2 == 0:
                            nc.scalar.copy(yst[:], po)
                        else:
                            nc.vector.tensor_copy(yst[:], po)
                        eng = (nc.sync if (tt == 0 or mb % 2 == 0)
                               else nc.gpsimd)
                        eng.dma_start(
                            out=arA_in[tt][mb * 128:(mb + 1) * 128, :],
                            in_=yst[:])
                    if no_collective:
                        nc.sync.dma_start(out=arA_out[tt],
                                          in_=arA_in[tt])
                    elif ar_mode == "full":
                        if tt == 1:
                            nc.gpsimd.collective_compute(
                                "AllReduce", ALU.add,
                                replica_groups=GROUPS,
                                ins=[arA_full[0].opt()],
                                outs=[arA_full[1].opt()])
                    else:
                        nc.gpsimd.collective_compute(
                            "AllReduce", ALU.add, replica_groups=GROUPS,
                            ins=[arA_in[tt].opt()],
                            outs=[arA_out[tt].opt()])

                # ---- emission schedule ----
                mark(f"qkv0_{l}")
                qkv_half(0)
                mark(f"attn0_{l}")
                attn_w(0, 0)
                attn_w(0, 1)
                mark(f"op0_{l}")
                op_half(0)                      # -> AR A(0)
                mark(f"fin_{l}")
                if pend_h1_final[0] is not None:
                    pend_h1_final[0]()          # resid M(l-1, h1)
                    pend_h1_final[0] = None

                # ======== MLP pool + AR buffers ========
                ml = tc.alloc_tile_pool(name=f"ml{_rep}_{l}", bufs=1,
                                        side="right")
                w1 = ml.tile([128, NCB, 2 * HIDS], BF16, name=f"w1{l}")
                w2 = ml.tile([128, NGB, C], BF16, name=f"w2{l}")
                nc.scalar.dma_start(
                    out=w1[:],
                    in_=w1_e.ap()[l].rearrange("(a p) m -> p a m",
                                               p=128))
                nc.scalar.dma_start(
                    out=w2[:],
                    in_=w2_e.ap()[l].rearrange("(a p) m -> p a m",
                                               p=128))
                if ar_mode == "full":
                    _mi = dram.tile([C, T], BF16, tag="ariMF",
                                    name=f"ariMF_{l}")
                    _mo = dram.tile([C, T], BF16, tag="aroMF",
                                    name=f"aroMF_{l}")
                    arM_in = [_mi[:, 0:512], _mi[:, 512:1024]]
                    arM_out = [_mo[:, 0:512], _mo[:, 512:1024]]
                    arM_full = (_mi, _mo)
                else:
                    arM_in = [dram.tile([C, 512], BF16, tag=f"ariM{tt}",
                                        name=f"ariM{tt}_{l}")[:]
                              for tt in range(2)]
                    arM_out = [dram.tile([C, 512], BF16,
                                         tag=f"aroM{tt}",
                                         name=f"aroM{tt}_{l}")[:]
                               for tt in range(2)]
                    arM_full = None
                gsb = [ml.tile([128, NGB, 512], BF16, name=f"gsb{i}")
                       for i in range(2)]
                asb = [ml.tile([128, NGB, 512], BF16, name=f"asb{i}")
                       for i in range(2)]

                def resid_half(tt, ar_out, tag, ar_in_=None):
                    if ar_mode == "nowait" and ar_in_ is not None:
                        ar_out = ar_in_
                    tsl = slice(tt * 512, (tt + 1) * 512)
                    for cb in range(NCB):
                        yf = ml.tile([128, 512], BF16, tag=f"yf{tag}",
                                     bufs=4, name=f"yf{tag}")
                        (nc.gpsimd if cb % 2 == 0
                         else nc.sync).dma_start(
                            out=yf[:],
                            in_=ar_out[cb * 128:(cb + 1) * 128, :])
                        nc.vector.tensor_add(xT[:, cb, tsl],
                                             xT[:, cb, tsl], yf[:])
                        if cb % 2 == 0:
                            nc.scalar.copy(xB[:, cb, tsl],
                                           xT[:, cb, tsl])
                        else:
                            nc.vector.tensor_copy(xB[:, cb, tsl],
                                                  xT[:, cb, tsl])

                def mlp_pu(tt, gsb_, asb_):
                    tsl = slice(tt * 512, (tt + 1) * 512)
                    sq = stats_sq(tt, "m")
                    pus = []

                    def pu_fill(gb):
                        pu = psum_rot(f"pu{gb}")[:, 0:512]
                        for cb in range(NCB):
                            nc.tensor.matmul(
                                pu,
                                w1[:, cb, gb * 128:(gb + 1) * 128],
                                xB[:, cb, tsl],
                                start=(cb == 0), stop=(cb == NCB - 1))
                        pus.append((gb, pu))

                    for gb in range(NGB):
                        pu_fill(gb)
                    big = stats_ssq(sq, "m", tt)
                    std_b = stats_std(ml, big, "m", tt)
                    for gb in range(NGB, 2 * NGB):
                        pu_fill(gb)
                    rstd2b = stats_rstd_b(ml, std_b, "m", tt)
                    for gb, pu in pus:
                        if gb < NGB:
                            nc.vector.tensor_mul(gsb_[:, gb, :], pu,
                                                 rstd2b[:])
                        else:
                            nc.vector.tensor_mul(asb_[:, gb - NGB, :],
                                                 pu, rstd2b[:])

                def mlp_act(tt, gsb_, asb_):
                    for gb in range(NGB):
                        nc.scalar.activation(asb_[:, gb, :],
                                             asb_[:, gb, :], AF.Silu)
                        nc.vector.tensor_mul(gsb_[:, gb, :],
                                             asb_[:, gb, :],
                                             gsb_[:, gb, :])

                def mlp_pz(tt, gsb_):
                    for mb in range(NCB):
                        pz = psum_rot(f"pz{mb}")[:, 0:512]
                        for kb in range(NGB):
                            nc.tensor.matmul(
                                pz,
                                w2[:, kb, mb * 128:(mb + 1) * 128],
                                gsb_[:, kb, :],
                                start=(kb == 0), stop=(kb == NGB - 1))
                        zst = ml.tile([128, 512], BF16, tag="zst",
                                      bufs=4, name="zst")
                        if tt == 0 or mb % 2 == 0:
                            nc.vector.tensor_copy(zst[:], pz)
                        else:
                            nc.scalar.copy(zst[:], pz)
                        eng = (nc.sync if (tt == 1 or mb % 2 == 0)
                               else nc.gpsimd)
                        eng.dma_start(
                            out=arM_in[tt][mb * 128:(mb + 1) * 128, :],
                            in_=zst[:])
                    if no_collective:
                        nc.sync.dma_start(out=arM_out[tt],
                                          in_=arM_in[tt])
                    elif ar_mode == "full":
                        if tt == 1:
                            nc.gpsimd.collective_compute(
                                "AllReduce", ALU.add,
                                replica_groups=GROUPS,
                                ins=[arM_full[0].opt()],
                                outs=[arM_full[1].opt()])
                    else:
                        nc.gpsimd.collective_compute(
                            "AllReduce", ALU.add, replica_groups=GROUPS,
                            ins=[arM_in[tt].opt()],
                            outs=[arM_out[tt].opt()])

                def make_final(ml_pool_, arM_out1_, arM_in1_):
                    def fin():
                        tsl = slice(512, 1024)
                        for cb in range(NCB):
                            zf = ml_pool_.tile([128, 512], BF16,
                                               tag="yfm", bufs=4,
                                               name="zf")
                            src_ = (arM_in1_ if ar_mode == "nowait"
                                    else arM_out1_)
                            (nc.gpsimd if cb % 2 == 0
                             else nc.sync).dma_start(
                                out=zf[:],
                                in_=src_[cb * 128:
                                         (cb + 1) * 128, :])
                            nc.vector.tensor_add(xT[:, cb, tsl],
                                                 xT[:, cb, tsl],
                                                 zf[:])
                            if cb % 2 == 0:
                                nc.scalar.copy(xB[:, cb, tsl],
                                               xT[:, cb, tsl])
                            else:
                                nc.vector.tensor_copy(xB[:, cb, tsl],
                                                      xT[:, cb, tsl])
                        ml_pool_.release()
                    return fin

                mark(f"qkv1_{l}")
                qkv_half(1)
                mark(f"attn1a_{l}")
                attn_w(1, 0)
                mark(f"attn1b_{l}")
                attn_w(1, 1)
                mark(f"op1_{l}")
                op_half(1)                      # -> AR A(1)
                at.release()
                mark(f"ra0_{l}")
                resid_half(0, arA_out[0], "a", arA_in[0])
                mark(f"mlp0_{l}")
                mlp_pu(0, gsb[0], asb[0])
                mlp_act(0, gsb[0], asb[0])
                mlp_pz(0, gsb[0])               # -> AR M(0)
                mark(f"ra1_{l}")
                resid_half(1, arA_out[1], "a", arA_in[1])
                mark(f"mlp1_{l}")
                mlp_pu(1, gsb[1], asb[1])
                mlp_act(1, gsb[1], asb[1])
                mlp_pz(1, gsb[1])               # -> AR M(1)
                mark(f"rm0_{l}")
                resid_half(0, arM_out[0], "m", arM_in[0])

                pend_h1_final[0] = make_final(ml, arM_out[1],
                                              arM_in[1])

            if pend_h1_final[0] is not None:
                pend_h1_final[0]()
                pend_h1_final[0] = None

            mark("lnf")
            # ---------------- ln_f -> xc (bf16) ----------------
            hd = tc.alloc_tile_pool(name=f"hd{_rep}", bufs=1)
            xc = hd.tile([128, NCB, T], BF16, name="xc")
            rstd_tok = hd.tile([128, 8], F32, name="rstdtok")
            hw = tc.alloc_tile_pool(name=f"hw{_rep}", bufs=1)
            NVT = VSH // 512            # 25
            VG = 2
            n_groups = (NVT + VG - 1) // VG
            wts = {}

            def load_wt(g):
                vts = list(range(g * VG, min((g + 1) * VG, NVT)))
                nv = len(vts)
                wt = hw.tile([128, NCB, VG * 512], BF16, tag="wt",
                             bufs=3, name=f"wt{g}")
                nc.scalar.dma_start(
                    out=wt[:, :, 0:nv * 512],
                    in_=wteT_e.ap()[:, vts[0] * 512:
                                    (vts[-1] + 1) * 512]
                        .rearrange("(a p) v -> p a v", p=128))
                wts[g] = (wt, vts, nv)

            load_wt(0)
            load_wt(1)

            hd0 = tc.alloc_tile_pool(name=f"hd0{_rep}", bufs=1)
            bigs = []
            for tt in range(NT):
                tsl = slice(tt * 512, (tt + 1) * 512)
                big = psum_big("mu_ms")
                mu_b = big[:, 0:512]
                for cb in range(NCB):
                    nc.tensor.matmul(mu_b, ones128b[:], xB[:, cb, tsl],
                                     start=(cb == 0),
                                     stop=(cb == NCB - 1))
                sqf = xpool.tile([128, NCB, 512], BF16, tag="sq",
                                 bufs=2, name="sqf")
                for cb in range(NCB):
                    if cb % 2 == 0:
                        nc.scalar.activation(sqf[:, cb, :],
                                             xT[:, cb, tsl], AF.Square)
                    else:
                        nc.vector.tensor_mul(sqf[:, cb, :],
                                             xT[:, cb, tsl],
                                             xT[:, cb, tsl])
                ms_b = big[:, 512:1024]
                for cb in range(NCB):
                    nc.tensor.matmul(ms_b, ones128b[:], sqf[:, cb, :],
                                     start=(cb == 0),
                                     stop=(cb == NCB - 1))
                bigs.append(big)
            for tt in range(NT):
                tsl = slice(tt * 512, (tt + 1) * 512)
                big = bigs[tt]
                mu_b, ms_b = big[:, 0:512], big[:, 512:1024]
                negmu = hd0.tile([128, 512], F32, tag=f"negmu{tt}",
                                 name="negmu")
                nc.scalar.activation(negmu[:], mu_b, AF.Copy,
                                     scale=-1.0 / C)
                mom2 = hd0.tile([128, 512], F32, tag=f"mom2{tt}",
                                name="mom2")
                nc.scalar.activation(mom2[:], ms_b, AF.Copy,
                                     scale=1.0 / C)
                mu2 = hd0.tile([128, 512], F32, tag=f"mu2{tt}",
                               name="mu2")
                nc.vector.tensor_mul(mu2[:], negmu[:], negmu[:])
                var = hd0.tile([128, 512], F32, tag=f"var{tt}",
                               name="var")
                nc.vector.tensor_sub(var[:], mom2[:], mu2[:])
                stdf = hd0.tile([128, 512], F32, tag=f"stdf{tt}",
                                name="stdf")
                nc.scalar.activation(stdf[:], var[:], AF.Sqrt, bias=EPS)
                # transpose std row -> per-token cols (bank B is free
                # once mom2 has read it); rstd applied at lm_head drain
                for tb4 in range(4):
                    nc.tensor.matmul(
                        big[:, 512 + tb4:513 + tb4],
                        stdf[0:1, tb4 * 128:(tb4 + 1) * 128],
                        ones_f[0:1, 0:1],
                        start=True, stop=True)
                nc.vector.reciprocal_approx_fast(
                    out=rstd_tok[:, tt * 4:tt * 4 + 4],
                    in_=big[:, 512:516])
                for cb in range(NCB):
                    nc.vector.tensor_add(xc[:, cb, tsl],
                                         xT[:, cb, tsl], negmu[:])
            hd0.release()

            mark("lmhead")
            # ---------------- lm_head (bf16) ----------------
            for g in range(n_groups):
                if g + 2 < n_groups:
                    load_wt(g + 2)
                wt, vts, nv = wts.pop(g)
                for tb in range(NSB):
                    po = 2 * (tb % 2)
                    lps = [sp.tile([128, 512], F32,
                                   tag=f"av{po + i}",
                                   name=f"lg{g}_{tb}_{i}")
                           for i in range(nv)]
                    for cb in range(NCB):
                        for i in range(nv):
                            nc.tensor.matmul(
                                lps[i][:],
                                xc[:, cb,
                                   tb * 128:(tb + 1) * 128],
                                wt[:, cb, i * 512:(i + 1) * 512],
                                start=(cb == 0),
                                stop=(cb == NCB - 1))
                    lo = hd.tile([128, VG * 512], BF16, tag="lo",
                                 bufs=3, name=f"lo{g}_{tb}")
                    for i in range(nv):
                        if i % 2 == 0:
                            nc.scalar.activation(
                                lo[:, i * 512:(i + 1) * 512],
                                lps[i][:], AF.Copy,
                                scale=rstd_tok[:, tb:tb + 1])
                        else:
                            nc.vector.tensor_scalar_mul(
                                lo[:, i * 512:(i + 1) * 512],
                                lps[i][:],
                                rstd_tok[:, tb:tb + 1])
                    nc.sync.dma_start(
                        out=logits_e[tb * 128:(tb + 1) * 128,
                                     vts[0] * 512:
                                     (vts[-1] + 1) * 512],
                        in_=lo[:, 0:nv * 512])
            hw.release()
            hd.release()

          xpool.release()

    nc.compile()
    return nc


def _prep_inputs(inputs):
    import ml_dtypes
    idx = np.asarray(inputs["idx"]).astype(np.int64)
    wte = np.asarray(inputs["wte"], np.float32)
    wpe = np.asarray(inputs["wpe"], np.float32)
    rms1 = np.asarray(inputs["rms1_w"], np.float32)
    rms2 = np.asarray(inputs["rms2_w"], np.float32)
    wq = np.asarray(inputs["wq"], np.float32)
    wk = np.asarray(inputs["wk"], np.float32)
    wv = np.asarray(inputs["wv"], np.float32)
    lq1 = np.asarray(inputs["lq1"], np.float32)
    lq2 = np.asarray(inputs["lq2"], np.float32)
    lk1 = np.asarray(inputs["lk1"], np.float32)
    lk2 = np.asarray(inputs["lk2"], np.float32)
    out_w = np.asarray(inputs["out_w"], np.float32)
    out_b = np.asarray(inputs["out_b"], np.float32)
    mlp_w1 = np.asarray(inputs["mlp_w1"], np.float32)
    mlp_b1 = np.asarray(inputs["mlp_b1"], np.float32)
    cproj_w = np.asarray(inputs["cproj_w"], np.float32)
    cproj_b = np.asarray(inputs["cproj_b"], np.float32)
    lnf_w = np.asarray(inputs["lnf_w"], np.float32)

    assert not (np.any(out_b) or np.any(mlp_b1) or np.any(cproj_b)), \
        "nonzero biases not supported by this kernel build"

    depth = np.arange(L, dtype=np.float32)
    lam_init = 0.8 - 0.6 * np.exp(-0.3 * (depth - 1.0))
    lam = (np.exp((lq1 * lk1).sum(-1)) - np.exp((lq2 * lk2).sum(-1))
           + lam_init[:, None])

    wteE = wte[:BLK] + wpe
    scale = 1.0 / math.sqrt(D)
    wq_f = wq * rms1[:, :, None, None] * scale
    wk_f = wk * rms1[:, :, None, None]
    wv_f = wv * rms1[:, :, None, None]
    w1_f = mlp_w1 * rms2[:, :, None]
    wteT_full = np.ascontiguousarray((wte * lnf_w[None, :]).T)

    jj = np.arange(512)[None, :]
    ppp = np.arange(128)[:, None]
    masks = np.zeros((128, 4, 2048), np.float32)
    for m in range(4):
        one = (jj >= (ppp + 128 * m)).astype(np.float32)
        masks[:, m, :] = np.tile(one, (1, 4))
    masks = masks.astype(ml_dtypes.bfloat16)

    in_maps = []
    for c in range(N_CORES):
        b, r = c // TP, c % TP
        hsl = slice(r * HPC, (r + 1) * HPC)
        x0 = np.ascontiguousarray(wteE[idx[b]].T)
        g0 = r * HIDS
        a0 = 2 * C + r * HIDS
        w1_s = np.concatenate(
            [w1_f[:, :, g0:g0 + HIDS], w1_f[:, :, a0:a0 + HIDS]],
            axis=2)
        w2_s = cproj_w[:, g0:g0 + HIDS, :]
        wteT_s = np.zeros((C, VSH), np.float32)
        lo, hi = r * VSH, min((r + 1) * VSH, V)
        if hi > lo:
            wteT_s[:, 0:hi - lo] = wteT_full[:, lo:hi]
        in_maps.append({
            "x0": x0,
            "x0b": x0.astype(ml_dtypes.bfloat16),
            "wq": np.ascontiguousarray(
                wq_f[:, :, hsl].reshape(L, C, HPC * HS)).astype(
                    ml_dtypes.bfloat16),
            "wk": np.ascontiguousarray(
                wk_f[:, :, hsl].reshape(L, C, HPC * HS)).astype(
                    ml_dtypes.bfloat16),
            "wv": np.ascontiguousarray(
                wv_f[:, :, hsl].reshape(L, C, HPC * HS)).astype(
                    ml_dtypes.bfloat16),
            "ow": np.ascontiguousarray(
                out_w.reshape(L, H, HS, C)[:, hsl].reshape(
                    L, HPC * HS, C)).astype(ml_dtypes.bfloat16),
            "w1": np.ascontiguousarray(w1_s).astype(ml_dtypes.bfloat16),
            "w2": np.ascontiguousarray(w2_s).astype(ml_dtypes.bfloat16),
            "neglam": np.ascontiguousarray(-lam[:, hsl]),
            "masks": masks,
            "wteT": wteT_s.astype(ml_dtypes.bfloat16),
        })
    return in_maps


def kernel(**inputs):
    from concourse.bass_utils import run_bass_kernel_spmd
    if "nc" not in _BUILT:
        _BUILT["nc"] = _build()
    nc = _BUILT["nc"]
    in_maps = _prep_inputs(inputs)
    res = run_bass_kernel_spmd(nc, in_maps, core_ids=list(range(N_CORES)))
    outs = []
    for b in range(B):
        parts = [res.results[b * TP + r]["logits"]
                 for r in range(TP)]
        outs.append(np.concatenate(parts, axis=1)[:, :V])
    return np.stack(outs, axis=0).astype(np.float32)
